# revision 19
# baseline (speedup 1.0000x reference)
"""DirectForce GNN message-passing kernel for 8 Trainium2 NeuronCores.

Structure
---------
Device (8 cores, edge-sharded, weights replicated):
    the edge MLP  mag_e = W3.(ssp(W2.(ssp(W1.x))))  for all E=262144 edges.
    The kernel is ACT-bound: softplus costs two table passes (exp, ln) per
    element on the scalar engine, which runs a flat 1 elem/cycle/partition
    regardless of dtype (measured), i.e. ~6.9us per 512-edge tile.  The
    design therefore minimizes ACT instruction count (4 wide instructions
    per tile over 4-bank PSUM tiles) and keeps every other engine under
    that roofline:

    * L1 runs feature-major: out = W1chunk.T @ x -> z1[feat, edge] in a
      single [128,4,512] PSUM tile (4 banks).  One EXP covers all 2048
      elements; the ShiftedSoftplus shift is folded into the LN pass as
      ln(0.5*e + 0.5) = ln(1+e^z) - ln 2  (scale/bias of the ACT op), so
      no bias folding on the host at all.
    * L2 runs TRANSPOSED: stationary operand is the h1 chunk, moving is
      W2, so z2 lands as [edge, feat] with edges on PSUM partitions.
      Again one 4-bank PSUM tile, one EXP, one LN.
    * L3 collapses to a per-partition dot: mag[e] = sum_j h2[e,j]*W3[j],
      one DVE scalar_tensor_tensor with accum_out per 128-edge chunk
      (sum over the free dim; b3 is added on the host).  No ones-matmul,
      no partition reduction, no staging copies.
    * Matmuls are fp16/bf16 (same PE rate as fp32r, half the DMA and
      LDWEIGHTS traffic); e-tiles are bf16 (exp can overflow fp16 range),
      h-tiles fp16.
    * Emission is software-pipelined one tile deep (L1(t+1) is issued
      between L1(t) and L2(t)) so the PE never waits on the softplus
      latency and both 4-bank PSUM pools (8 banks total) single-buffer
      without stalls.

Host (index work + O(E) reductions, ~0.3% of the FLOPs):
    the category/key lexsort pairing (exact transcription of the
    reference), magnitude symmetrization with the paired reverse edge,
    and the [N,3] segment-sum of mag * unit_vec over center atoms.

Hardware constraint that shapes the emission: every TPB instruction encodes
at most ONE semaphore wait (NEURON_ISA_TPB_EVENTS has a single wait slot).
Tile emits multi-wait instructions freely, so after scheduling we legalize:
every excess wait is hoisted onto a NOP inserted just before the offending
instruction on the same engine -- sound because each engine's sequencer
executes waits in program order.
"""

import numpy as np

E = 262144
D = 512
N_CORES = 8
RPC = E // N_CORES          # rows (edges) per core = 32768
RT = 512                    # rows per tile iteration
NT = RPC // RT              # 64 row-tiles per core
KC = D // 128               # 4 contraction chunks

_CACHE = {}


def _legalize_waits(nc):
    """Every TPB instruction carries at most one sync wait; hoist extras onto
    same-engine NOPs placed immediately before the offender."""
    import concourse.mybir as mybir

    eng_map = {
        mybir.EngineType.PE: nc.tensor,
        mybir.EngineType.Activation: nc.scalar,
        mybir.EngineType.DVE: nc.vector,
        mybir.EngineType.Pool: nc.gpsimd,
        mybir.EngineType.SP: nc.sync,
    }
    hoist_all = getattr(nc, "_hoist_all_wait_ids", frozenset())
    n_nops = 0
    for blk in nc.main_func.blocks:
        offenders = [
            ins for ins in blk.instructions
            if ins.sync_info is not None and (
                len(ins.sync_info.on_wait) > 1
                # Matmuls lower to LDWEIGHTS+MATMUL; if the wait rides on the
                # MATMUL, the LDWEIGHTS can read a still-being-written
                # stationary operand (the L2 matmuls' stationary is h1,
                # produced by the LN moments earlier).  For those, hoist ALL
                # waits onto preceding NOPs so they execute before the
                # weight load.
                or (len(ins.sync_info.on_wait) >= 1 and id(ins) in hoist_all)
            )
        ]
        for ins in offenders:
            si = ins.sync_info
            waits = list(si.on_wait)
            if id(ins) in hoist_all:
                si.on_wait = []
            else:
                si.on_wait = [waits[-1]]
                waits = waits[:-1]
            eng = eng_map.get(ins.engine, nc.sync)
            idx = blk.instructions.index(ins)
            for w in waits:
                nop_ins = eng.nop(nofuse=True).ins
                nop_ins.sync_info = mybir.SyncInfo(on_wait=[w], on_update=[])
                # nop() appended it to the current bb; move it before `ins`
                cur = nc.cur_bb.bb
                cur.instructions.remove(nop_ins)
                blk.instructions.insert(idx, nop_ins)
                idx += 1
                n_nops += 1
    return n_nops


def _build_program(zero_b1=True, zero_b2=True):
    import concourse.bass as bass
    import concourse.mybir as mybir
    import concourse.tile as tile

    f32 = mybir.dt.float32
    f16 = mybir.dt.float16
    bf16 = mybir.dt.bfloat16
    AF = mybir.ActivationFunctionType
    OP = mybir.AluOpType

    nc = bass.Bass()
    xt = nc.dram_tensor("xt", [D, RPC], f16, kind="ExternalInput")
    w1p = nc.dram_tensor("w1p", [128, KC, D], f16, kind="ExternalInput")
    w2p = nc.dram_tensor("w2p", [128, KC, D], f16, kind="ExternalInput")
    w3b = nc.dram_tensor("w3b", [128, D], f16, kind="ExternalInput")
    b1p = nc.dram_tensor("b1p", [128, KC], f32, kind="ExternalInput")
    b2r = nc.dram_tensor("b2r", [1, D], f16, kind="ExternalInput")
    # mag for edge  t*RT + c*128 + p  lives at mag[p, 4*t + c]
    mag = nc.dram_tensor("mag", [128, KC * NT], f32, kind="ExternalOutput")

    xt_v = xt.rearrange("(c p) r -> p c r", p=128)  # [128, KC, RPC]

    with tile.TileContext(nc) as tc:
        with (
            tc.tile_pool(name="singles", bufs=1) as singles,
            tc.tile_pool(name="xp", bufs=3) as xp,
            tc.tile_pool(name="e2p", bufs=2) as e2p,
            tc.tile_pool(name="e1p", bufs=2) as e1p,
            tc.tile_pool(name="h1p", bufs=2) as h1p,
            tc.tile_pool(name="h2p", bufs=2) as h2p,
            tc.tile_pool(name="prodp", bufs=2) as prodp,
            tc.tile_pool(name="ps1p", bufs=1, space="PSUM") as ps1p,
            tc.tile_pool(name="ps2p", bufs=1, space="PSUM") as ps2p,
        ):
            w1t = singles.tile([128, KC, D], f16)
            nc.sync.dma_start(out=w1t, in_=w1p[:, :, :])
            w2t = singles.tile([128, KC, D], f16)
            nc.sync.dma_start(out=w2t, in_=w2p[:, :, :])
            w3t = singles.tile([128, D], f16)
            nc.sync.dma_start(out=w3t, in_=w3b[:, :])
            half_t = singles.tile([128, 1], f32)
            nc.vector.memset(half_t, 0.5)
            if not zero_b1:
                b1t = singles.tile([128, KC], f32)
                nc.sync.dma_start(out=b1t, in_=b1p[:, :])
            if not zero_b2:
                b2t = singles.tile([1, D], f16)
                nc.sync.dma_start(out=b2t, in_=b2r[:, :])
                ones1 = singles.tile([1, 128], f16)
                nc.vector.memset(ones1, 1.0)
            magsb = singles.tile([128, KC * NT], f32)

            l2_ids = set()
            h1_prev = None
            for t in range(NT + 1):
                if t < NT:
                    # ---- L1(t): z1[feat, edge] = W1.T @ x
                    x_t = xp.tile([128, KC, RT], f16, tag="x")
                    nc.sync.dma_start(out=x_t,
                                      in_=xt_v[:, :, t * RT:(t + 1) * RT])
                    ps1 = ps1p.tile([128, KC, RT], f32, tag="ps1")
                    for jc in range(KC):
                        for dc in range(KC):
                            r = nc.tensor.matmul(
                                ps1[:, jc, :],
                                w1t[:, dc, jc * 128:(jc + 1) * 128],
                                x_t[:, dc, :],
                                start=(dc == 0), stop=(dc == KC - 1),
                            )
                            if t == 0 and jc == 0 and dc == 0:
                                # tile 0: the w1t DMA is still in flight; the
                                # first LDWEIGHTS must not outrun its wait
                                l2_ids.add(id(r.ins))
                    e1 = e1p.tile([128, KC, RT], bf16, tag="e1")
                    if zero_b1 and t == 0:
                        nc.scalar.activation(e1[:, 0:2, :], ps1[:, 0:2, :],
                                             AF.Exp)
                        nc.scalar.activation(e1[:, 2:4, :], ps1[:, 2:4, :],
                                             AF.Exp)
                    elif zero_b1:
                        nc.scalar.activation(e1, ps1, AF.Exp)
                    else:
                        for jc in range(KC):
                            nc.scalar.activation(e1[:, jc, :], ps1[:, jc, :],
                                                 AF.Exp, bias=b1t[:, jc:jc + 1])
                    h1 = h1p.tile([128, KC, RT], f16, tag="h1")
                    # ssp(z) = ln(0.5*e^z + 0.5)
                    nc.scalar.activation(h1, e1, AF.Ln, bias=half_t[:, 0:1], scale=0.5)

                if t >= 1:
                    # ---- L2(t-1), transposed: z2[edge, feat] = h1chunk.T @ W2
                    tm = t - 1
                    h1m = h1_prev
                    ps2 = ps2p.tile([128, KC, RT], f32, tag="ps2")
                    for c in range(KC):
                        if not zero_b2:
                            nc.tensor.matmul(ps2[:, c, :], ones1, b2t,
                                             start=True, stop=False)
                        for kc in range(KC):
                            r = nc.tensor.matmul(
                                ps2[:, c, :],
                                h1m[:, kc, c * 128:(c + 1) * 128],
                                w2t[:, kc, :],
                                start=(kc == 0 and zero_b2),
                                stop=(kc == KC - 1),
                            )
                            # Only the first matmul of the L2 block can have
                            # its LDWEIGHTS outrun the h1 producer: the shadow
                            # weight buffer is depth-1, so every later LDW
                            # issues after the previous instruction (and hence
                            # after the hoisted wait) has started.
                            if c == 0 and kc == 0:
                                l2_ids.add(id(r.ins))
                    e2 = e2p.tile([128, KC, RT], bf16, tag="e2")
                    if tm == NT - 1:
                        # last tile: halved EXP shortens the drain chain
                        nc.scalar.activation(e2[:, 0:2, :], ps2[:, 0:2, :],
                                             AF.Exp)
                        nc.scalar.activation(e2[:, 2:4, :], ps2[:, 2:4, :],
                                             AF.Exp)
                    else:
                        nc.scalar.activation(e2, ps2, AF.Exp)
                    h2 = h2p.tile([128, KC, RT], f16, tag="h2")
                    nc.scalar.activation(h2, e2, AF.Ln,
                                         bias=half_t[:, 0:1], scale=0.5)
                    # ---- L3: mag[e] = sum_j h2[e, j] * W3[j]  (b3 on host)
                    for c in range(KC):
                        prod = prodp.tile([128, RT], f16, tag="prod")
                        nc.vector.scalar_tensor_tensor(
                            prod, h2[:, c, :], 1.0, w3t,
                            OP.mult, OP.mult,
                            accum_out=magsb[:, KC * tm + c:KC * tm + c + 1],
                        )

                if t < NT:
                    h1_prev = h1

            nc.sync.dma_start(out=mag[:, :], in_=magsb)

    nc._hoist_all_wait_ids = frozenset(l2_ids)
    _legalize_waits(nc)
    return nc


def _get_program(zero_b1=True, zero_b2=True):
    key = (zero_b1, zero_b2)
    if key not in _CACHE:
        _CACHE[key] = _build_program(zero_b1, zero_b2)
    return _CACHE[key]


def _run_mlp(edge_emb, W1, b1, W2, b2, W3, b3, trace=False):
    """Run the edge MLP on 8 NeuronCores; returns mag [E] fp32 (incl. b3)."""
    from concourse.bass_utils import run_bass_kernel_spmd

    W1 = np.asarray(W1, np.float32)
    W2 = np.asarray(W2, np.float32)
    W3 = np.asarray(W3, np.float32)
    b1 = np.asarray(b1, np.float32)
    b2 = np.asarray(b2, np.float32)
    b3 = np.asarray(b3, np.float32)

    zb1 = not np.any(b1)
    zb2 = not np.any(b2)
    nc = _get_program(zb1, zb2)

    # w1p[p, dc, j] = W1[dc*128+p, j];  w2p[p, kc, j] = W2[kc*128+p, j]
    w1p = np.ascontiguousarray(
        W1.reshape(KC, 128, D).transpose(1, 0, 2).astype(np.float16))
    w2p = np.ascontiguousarray(
        W2.reshape(KC, 128, D).transpose(1, 0, 2).astype(np.float16))
    w3bc = np.ascontiguousarray(
        np.broadcast_to(W3[:, 0].astype(np.float16)[None, :], (128, D)))
    b1pk = np.ascontiguousarray(b1.reshape(KC, 128).T.astype(np.float32))
    b2rw = b2.astype(np.float16)[None, :]

    emb = np.asarray(edge_emb, np.float32)
    in_maps = []
    for c in range(N_CORES):
        shard = emb[c * RPC:(c + 1) * RPC, :]
        xt_shard = np.ascontiguousarray(shard.T).astype(np.float16)
        in_maps.append({"xt": xt_shard, "w1p": w1p, "w2p": w2p, "w3b": w3bc,
                        "b1p": b1pk, "b2r": b2rw})

    kwargs = {}
    if trace:
        _register_ntff_hook()
        kwargs["trace"] = True
    res = run_bass_kernel_spmd(nc, in_maps, core_ids=list(range(N_CORES)),
                               **kwargs)
    shards = []
    for c in range(N_CORES):
        m = res.results[c]["mag"]            # [128, KC*NT], col = 4*t + c
        # edge t*RT + c*128 + p  ->  [t, c, p] order
        shards.append(np.ascontiguousarray(
            m.reshape(128, NT, KC).transpose(1, 2, 0)).reshape(-1))
    mag_out = np.concatenate(shards)
    if trace:
        print(f"HW exec time: {res.exec_time_ns} ns "
              f"(mean {res.mean_exec_time_ns} ns across cores)")
    return mag_out + np.float32(b3[0])


def _register_ntff_hook():
    """The image's antenv lacks axon_hooks; synthesize it so trace=True can
    capture NTFF profiles through the axon PJRT library."""
    import sys, types
    if "antenv.axon_hooks" in sys.modules:
        return
    mod = types.ModuleType("antenv.axon_hooks")
    state = {"hook": None}
    mod.set_axon_ntff_profile_hook = lambda h: state.__setitem__("hook", h)
    mod.get_axon_ntff_profile_hook = lambda: state["hook"]
    sys.modules["antenv.axon_hooks"] = mod
    import antenv
    antenv.axon_hooks = mod
    try:
        from trn_agent_boot.trn_boot import _ntff_profile_via_ctypes
        mod.set_axon_ntff_profile_hook(
            _ntff_profile_via_ctypes("/opt/axon/libaxon_pjrt.so"))
    except Exception:
        pass


def _forces_from_mag(mag, edge_vectors, edge_lengths, edge_index,
                     edge_cell_shift, N):
    """Exact numpy transcription of the reference pairing + segment sum."""
    uv = np.asarray(edge_vectors, np.float32) / np.asarray(
        edge_lengths, np.float32)[:, None]
    s = np.asarray(edge_cell_shift, np.int64)
    s0, s1, s2 = s[:, 0], s[:, 1], s[:, 2]
    c = np.asarray(edge_index[0], np.int64)
    n = np.asarray(edge_index[1], np.int64)
    fwd = c * N + n
    rev = n * N + c
    N2 = N * N
    conds = [
        (s0 == 0) & (s1 == 0) & (s2 == 0),
        (s0 == -1) & (s1 == 0) & (s2 == 0),
        (s1 == -1) & (s2 == 0),
        (s2 == -1),
        (s0 == 1) & (s1 == 0) & (s2 == 0),
        (s1 == 1) & (s2 == 0),
        (s2 == 1),
    ]
    keys = [
        fwd,
        fwd,
        (s0 + 2) * N2 + fwd,
        (s0 + 6) * (s1 + 2) * N2 + fwd,
        rev,
        (-s0 + 2) * N2 + rev,
        (-s0 + 6) * (-s1 + 2) * N2 + rev,
    ]
    cat = np.select(conds, [np.full_like(c, i) for i in range(7)],
                    np.full_like(c, 6))
    key = np.select(conds, keys, rev)
    perm = np.lexsort((key, cat))
    mag_s = mag[perm]
    uv_s = uv[perm]
    c_s = c[perm]
    n_s = n[perm]
    cat_s = cat[perm]
    perm2 = np.lexsort((n_s * N + c_s, cat_s))
    M = int(np.sum((cat_s >= 1) & (cat_s <= 3)))
    idx = np.arange(E, dtype=np.int64)
    partner = np.where(cat_s == 0, perm2,
                       np.where(cat_s <= 3, idx + M, idx - M))
    mag_f = (mag_s + mag_s[partner]) * np.float32(0.5)
    contrib = mag_f[:, None] * uv_s
    forces = np.empty((N, 3), np.float32)
    for d in range(3):
        forces[:, d] = np.bincount(c_s, weights=contrib[:, d],
                                   minlength=N).astype(np.float32)
    return forces


def kernel(edge_emb, edge_vectors, edge_lengths, W1, b1, W2, b2, W3, b3,
           edge_index, edge_cell_shift, atom_count, _trace=False):
    N = int(atom_count)
    mag = _run_mlp(edge_emb, W1, b1, W2, b2, W3, b3, trace=_trace)
    return _forces_from_mag(mag, edge_vectors, edge_lengths, edge_index,
                            edge_cell_shift, N)


# revision 21
# speedup vs baseline: 1.0003x; 1.0003x over previous
"""DirectForce GNN message-passing kernel for 8 Trainium2 NeuronCores.

Structure
---------
Device (8 cores, edge-sharded, weights replicated):
    the edge MLP  mag_e = W3.(ssp(W2.(ssp(W1.x))))  for all E=262144 edges.
    The kernel is ACT-bound: softplus costs two table passes (exp, ln) per
    element on the scalar engine, which runs a flat 1 elem/cycle/partition
    regardless of dtype (measured), i.e. ~6.9us per 512-edge tile.  The
    design therefore minimizes ACT instruction count (4 wide instructions
    per tile over 4-bank PSUM tiles) and keeps every other engine under
    that roofline:

    * L1 runs feature-major: out = W1chunk.T @ x -> z1[feat, edge] in a
      single [128,4,512] PSUM tile (4 banks).  One EXP covers all 2048
      elements; the ShiftedSoftplus shift is folded into the LN pass as
      ln(0.5*e + 0.5) = ln(1+e^z) - ln 2  (scale/bias of the ACT op), so
      no bias folding on the host at all.
    * L2 runs TRANSPOSED: stationary operand is the h1 chunk, moving is
      W2, so z2 lands as [edge, feat] with edges on PSUM partitions.
      Again one 4-bank PSUM tile, one EXP, one LN.
    * L3 collapses to a per-partition dot: mag[e] = sum_j h2[e,j]*W3[j],
      one DVE scalar_tensor_tensor with accum_out per 128-edge chunk
      (sum over the free dim; b3 is added on the host).  No ones-matmul,
      no partition reduction, no staging copies.
    * Matmuls are fp16/bf16 (same PE rate as fp32r, half the DMA and
      LDWEIGHTS traffic); e-tiles are bf16 (exp can overflow fp16 range),
      h-tiles fp16.
    * Emission is software-pipelined one tile deep (L1(t+1) is issued
      between L1(t) and L2(t)) so the PE never waits on the softplus
      latency and both 4-bank PSUM pools (8 banks total) single-buffer
      without stalls.

Host (index work + O(E) reductions, ~0.3% of the FLOPs):
    the category/key lexsort pairing (exact transcription of the
    reference), magnitude symmetrization with the paired reverse edge,
    and the [N,3] segment-sum of mag * unit_vec over center atoms.

Hardware constraint that shapes the emission: every TPB instruction encodes
at most ONE semaphore wait (NEURON_ISA_TPB_EVENTS has a single wait slot).
Tile emits multi-wait instructions freely, so after scheduling we legalize:
every excess wait is hoisted onto a NOP inserted just before the offending
instruction on the same engine -- sound because each engine's sequencer
executes waits in program order.
"""

import numpy as np

E = 262144
D = 512
N_CORES = 8
RPC = E // N_CORES          # rows (edges) per core = 32768
RT = 512                    # rows per tile iteration
NT = RPC // RT              # 64 row-tiles per core
KC = D // 128               # 4 contraction chunks

_CACHE = {}


def _legalize_waits(nc):
    """Every TPB instruction carries at most one sync wait; hoist extras onto
    same-engine NOPs placed immediately before the offender."""
    import concourse.mybir as mybir

    eng_map = {
        mybir.EngineType.PE: nc.tensor,
        mybir.EngineType.Activation: nc.scalar,
        mybir.EngineType.DVE: nc.vector,
        mybir.EngineType.Pool: nc.gpsimd,
        mybir.EngineType.SP: nc.sync,
    }
    hoist_all = getattr(nc, "_hoist_all_wait_ids", frozenset())
    n_nops = 0
    for blk in nc.main_func.blocks:
        offenders = [
            ins for ins in blk.instructions
            if ins.sync_info is not None and (
                len(ins.sync_info.on_wait) > 1
                # Matmuls lower to LDWEIGHTS+MATMUL; if the wait rides on the
                # MATMUL, the LDWEIGHTS can read a still-being-written
                # stationary operand (the L2 matmuls' stationary is h1,
                # produced by the LN moments earlier).  For those, hoist ALL
                # waits onto preceding NOPs so they execute before the
                # weight load.
                or (len(ins.sync_info.on_wait) >= 1 and id(ins) in hoist_all)
            )
        ]
        for ins in offenders:
            si = ins.sync_info
            waits = list(si.on_wait)
            if id(ins) in hoist_all:
                si.on_wait = []
            else:
                si.on_wait = [waits[-1]]
                waits = waits[:-1]
            eng = eng_map.get(ins.engine, nc.sync)
            idx = blk.instructions.index(ins)
            for w in waits:
                nop_ins = eng.nop(nofuse=True).ins
                nop_ins.sync_info = mybir.SyncInfo(on_wait=[w], on_update=[])
                # nop() appended it to the current bb; move it before `ins`
                cur = nc.cur_bb.bb
                cur.instructions.remove(nop_ins)
                blk.instructions.insert(idx, nop_ins)
                idx += 1
                n_nops += 1
    return n_nops


def _build_program(zero_b1=True, zero_b2=True):
    import concourse.bass as bass
    import concourse.mybir as mybir
    import concourse.tile as tile

    f32 = mybir.dt.float32
    f16 = mybir.dt.float16
    bf16 = mybir.dt.bfloat16
    AF = mybir.ActivationFunctionType
    OP = mybir.AluOpType

    nc = bass.Bass()
    xt = nc.dram_tensor("xt", [D, RPC], f16, kind="ExternalInput")
    w1p = nc.dram_tensor("w1p", [128, KC, D], f16, kind="ExternalInput")
    w2p = nc.dram_tensor("w2p", [128, KC, D], f16, kind="ExternalInput")
    w3b = nc.dram_tensor("w3b", [128, D], f16, kind="ExternalInput")
    b1p = nc.dram_tensor("b1p", [128, KC], f32, kind="ExternalInput")
    b2r = nc.dram_tensor("b2r", [1, D], f16, kind="ExternalInput")
    # mag for edge  t*RT + c*128 + p  lives at mag[p, 4*t + c]
    mag = nc.dram_tensor("mag", [128, KC * NT], f32, kind="ExternalOutput")

    xt_v = xt.rearrange("(c p) r -> p c r", p=128)  # [128, KC, RPC]

    with tile.TileContext(nc) as tc:
        with (
            tc.tile_pool(name="singles", bufs=1) as singles,
            tc.tile_pool(name="xp", bufs=3) as xp,
            tc.tile_pool(name="e2p", bufs=2) as e2p,
            tc.tile_pool(name="e1p", bufs=2) as e1p,
            tc.tile_pool(name="h1p", bufs=2) as h1p,
            tc.tile_pool(name="h2p", bufs=2) as h2p,
            tc.tile_pool(name="prodp", bufs=2) as prodp,
            tc.tile_pool(name="ps1p", bufs=1, space="PSUM") as ps1p,
            tc.tile_pool(name="ps2p", bufs=1, space="PSUM") as ps2p,
        ):
            w1t = singles.tile([128, KC, D], f16)
            nc.sync.dma_start(out=w1t, in_=w1p[:, :, :])
            w2t = singles.tile([128, KC, D], f16)
            nc.sync.dma_start(out=w2t, in_=w2p[:, :, :])
            w3t = singles.tile([128, D], f16)
            nc.sync.dma_start(out=w3t, in_=w3b[:, :])
            half_t = singles.tile([128, 1], f32)
            nc.vector.memset(half_t, 0.5)
            if not zero_b1:
                b1t = singles.tile([128, KC], f32)
                nc.sync.dma_start(out=b1t, in_=b1p[:, :])
            if not zero_b2:
                b2t = singles.tile([1, D], f16)
                nc.sync.dma_start(out=b2t, in_=b2r[:, :])
                ones1 = singles.tile([1, 128], f16)
                nc.vector.memset(ones1, 1.0)
            magsb = singles.tile([128, KC * NT], f32)

            l2_ids = set()
            h1_prev = None
            for t in range(NT + 1):
                if t < NT:
                    # ---- L1(t): z1[feat, edge] = W1.T @ x
                    x_t = xp.tile([128, KC, RT], f16, tag="x")
                    nc.sync.dma_start(out=x_t,
                                      in_=xt_v[:, :, t * RT:(t + 1) * RT])
                    ps1 = ps1p.tile([128, KC, RT], f32, tag="ps1")
                    for jc in range(KC):
                        for dc in range(KC):
                            r = nc.tensor.matmul(
                                ps1[:, jc, :],
                                w1t[:, dc, jc * 128:(jc + 1) * 128],
                                x_t[:, dc, :],
                                start=(dc == 0), stop=(dc == KC - 1),
                            )
                            if t == 0 and jc == 0 and dc == 0:
                                # tile 0: the w1t DMA is still in flight; the
                                # first LDWEIGHTS must not outrun its wait
                                l2_ids.add(id(r.ins))
                    e1 = e1p.tile([128, KC, RT], bf16, tag="e1")
                    if zero_b1 and t == 0:
                        nc.scalar.activation(e1[:, 0:2, :], ps1[:, 0:2, :],
                                             AF.Exp)
                        nc.scalar.activation(e1[:, 2:4, :], ps1[:, 2:4, :],
                                             AF.Exp)
                    elif zero_b1:
                        nc.scalar.activation(e1, ps1, AF.Exp)
                    else:
                        for jc in range(KC):
                            nc.scalar.activation(e1[:, jc, :], ps1[:, jc, :],
                                                 AF.Exp, bias=b1t[:, jc:jc + 1])
                    h1 = h1p.tile([128, KC, RT], f16, tag="h1")
                    # ssp(z) = ln(0.5*e^z + 0.5)
                    nc.scalar.activation(h1, e1, AF.Ln, bias=half_t[:, 0:1], scale=0.5)

                if t >= 1:
                    # ---- L2(t-1), transposed: z2[edge, feat] = h1chunk.T @ W2
                    tm = t - 1
                    h1m = h1_prev
                    ps2 = ps2p.tile([128, KC, RT], f32, tag="ps2")
                    for c in range(KC):
                        if not zero_b2:
                            nc.tensor.matmul(ps2[:, c, :], ones1, b2t,
                                             start=True, stop=False)
                        for kc in range(KC):
                            r = nc.tensor.matmul(
                                ps2[:, c, :],
                                h1m[:, kc, c * 128:(c + 1) * 128],
                                w2t[:, kc, :],
                                start=(kc == 0 and zero_b2),
                                stop=(kc == KC - 1),
                            )
                            # Only the first matmul of the L2 block can have
                            # its LDWEIGHTS outrun the h1 producer: the shadow
                            # weight buffer is depth-1, so every later LDW
                            # issues after the previous instruction (and hence
                            # after the hoisted wait) has started.
                            if c == 0 and kc == 0:
                                l2_ids.add(id(r.ins))
                    e2 = e2p.tile([128, KC, RT], bf16, tag="e2")
                    if tm == NT - 1:
                        # last tile: halved EXP shortens the drain chain
                        nc.scalar.activation(e2[:, 0:2, :], ps2[:, 0:2, :],
                                             AF.Exp)
                        nc.scalar.activation(e2[:, 2:4, :], ps2[:, 2:4, :],
                                             AF.Exp)
                    else:
                        nc.scalar.activation(e2, ps2, AF.Exp)
                    h2 = h2p.tile([128, KC, RT], f16, tag="h2")
                    nc.scalar.activation(h2, e2, AF.Ln,
                                         bias=half_t[:, 0:1], scale=0.5)
                    # ---- L3: mag[e] = sum_j h2[e, j] * W3[j]  (b3 on host)
                    for c in range(KC):
                        prod = prodp.tile([128, RT], f16, tag="prod")
                        nc.vector.scalar_tensor_tensor(
                            prod, h2[:, c, :], 1.0, w3t,
                            OP.mult, OP.mult,
                            accum_out=magsb[:, KC * tm + c:KC * tm + c + 1],
                        )

                if t < NT:
                    h1_prev = h1

            nc.sync.dma_start(out=mag[:, :], in_=magsb)

    nc._hoist_all_wait_ids = frozenset(l2_ids)
    _legalize_waits(nc)
    return nc


def _get_program(zero_b1=True, zero_b2=True):
    key = (zero_b1, zero_b2)
    if key not in _CACHE:
        _CACHE[key] = _build_program(zero_b1, zero_b2)
    return _CACHE[key]


def _run_mlp(edge_emb, W1, b1, W2, b2, W3, b3, trace=False):
    """Run the edge MLP on 8 NeuronCores; returns mag [E] fp32 (incl. b3)."""
    from concourse.bass_utils import run_bass_kernel_spmd

    W1 = np.asarray(W1, np.float32)
    W2 = np.asarray(W2, np.float32)
    W3 = np.asarray(W3, np.float32)
    b1 = np.asarray(b1, np.float32)
    b2 = np.asarray(b2, np.float32)
    b3 = np.asarray(b3, np.float32)

    zb1 = not np.any(b1)
    zb2 = not np.any(b2)
    nc = _get_program(zb1, zb2)

    # w1p[p, dc, j] = W1[dc*128+p, j];  w2p[p, kc, j] = W2[kc*128+p, j]
    w1p = np.ascontiguousarray(
        W1.reshape(KC, 128, D).transpose(1, 0, 2).astype(np.float16))
    w2p = np.ascontiguousarray(
        W2.reshape(KC, 128, D).transpose(1, 0, 2).astype(np.float16))
    w3bc = np.ascontiguousarray(
        np.broadcast_to(W3[:, 0].astype(np.float16)[None, :], (128, D)))
    b1pk = np.ascontiguousarray(b1.reshape(KC, 128).T.astype(np.float32))
    b2rw = b2.astype(np.float16)[None, :]

    emb = np.asarray(edge_emb, np.float32)
    in_maps = []
    for c in range(N_CORES):
        shard = emb[c * RPC:(c + 1) * RPC, :]
        xt_shard = np.ascontiguousarray(shard.T).astype(np.float16)
        in_maps.append({"xt": xt_shard, "w1p": w1p, "w2p": w2p, "w3b": w3bc,
                        "b1p": b1pk, "b2r": b2rw})

    kwargs = {}
    if trace:
        _register_ntff_hook()
        kwargs["trace"] = True
    res = run_bass_kernel_spmd(nc, in_maps, core_ids=list(range(N_CORES)),
                               **kwargs)
    shards = []
    for c in range(N_CORES):
        m = res.results[c]["mag"]            # [128, KC*NT], col = 4*t + c
        # edge t*RT + c*128 + p  ->  [t, c, p] order
        shards.append(np.ascontiguousarray(
            m.reshape(128, NT, KC).transpose(1, 2, 0)).reshape(-1))
    mag_out = np.concatenate(shards)
    if trace:
        print(f"HW exec time: {res.exec_time_ns} ns "
              f"(mean {res.mean_exec_time_ns} ns across cores)")
    return mag_out + np.float32(b3[0])


def _register_ntff_hook():
    """The image's antenv lacks axon_hooks; synthesize it so trace=True can
    capture NTFF profiles through the axon PJRT library."""
    import sys, types
    if "antenv.axon_hooks" in sys.modules:
        return
    mod = types.ModuleType("antenv.axon_hooks")
    state = {"hook": None}
    mod.set_axon_ntff_profile_hook = lambda h: state.__setitem__("hook", h)
    mod.get_axon_ntff_profile_hook = lambda: state["hook"]
    sys.modules["antenv.axon_hooks"] = mod
    import antenv
    antenv.axon_hooks = mod
    try:
        from trn_agent_boot.trn_boot import _ntff_profile_via_ctypes
        mod.set_axon_ntff_profile_hook(
            _ntff_profile_via_ctypes("/opt/axon/libaxon_pjrt.so"))
    except Exception:
        pass


def _forces_from_mag(mag, edge_vectors, edge_lengths, edge_index,
                     edge_cell_shift, N):
    """Exact numpy transcription of the reference pairing + segment sum."""
    uv = np.asarray(edge_vectors, np.float32) / np.asarray(
        edge_lengths, np.float32)[:, None]
    s = np.asarray(edge_cell_shift, np.int64)
    s0, s1, s2 = s[:, 0], s[:, 1], s[:, 2]
    c = np.asarray(edge_index[0], np.int64)
    n = np.asarray(edge_index[1], np.int64)
    fwd = c * N + n
    rev = n * N + c
    N2 = N * N
    conds = [
        (s0 == 0) & (s1 == 0) & (s2 == 0),
        (s0 == -1) & (s1 == 0) & (s2 == 0),
        (s1 == -1) & (s2 == 0),
        (s2 == -1),
        (s0 == 1) & (s1 == 0) & (s2 == 0),
        (s1 == 1) & (s2 == 0),
        (s2 == 1),
    ]
    keys = [
        fwd,
        fwd,
        (s0 + 2) * N2 + fwd,
        (s0 + 6) * (s1 + 2) * N2 + fwd,
        rev,
        (-s0 + 2) * N2 + rev,
        (-s0 + 6) * (-s1 + 2) * N2 + rev,
    ]
    cat = np.select(conds, [np.full_like(c, i) for i in range(7)],
                    np.full_like(c, 6))
    key = np.select(conds, keys, rev)
    perm = np.lexsort((key, cat))
    mag_s = mag[perm]
    uv_s = uv[perm]
    c_s = c[perm]
    n_s = n[perm]
    cat_s = cat[perm]
    perm2 = np.lexsort((n_s * N + c_s, cat_s))
    M = int(np.sum((cat_s >= 1) & (cat_s <= 3)))
    idx = np.arange(E, dtype=np.int64)
    partner = np.where(cat_s == 0, perm2,
                       np.where(cat_s <= 3, idx + M, idx - M))
    mag_f = (mag_s + mag_s[partner]) * np.float32(0.5)
    contrib = mag_f[:, None] * uv_s
    forces = np.empty((N, 3), np.float32)
    for d in range(3):
        forces[:, d] = np.bincount(c_s, weights=contrib[:, d],
                                   minlength=N).astype(np.float32)
    return forces


def kernel(edge_emb, edge_vectors, edge_lengths, W1, b1, W2, b2, W3, b3,
           edge_index, edge_cell_shift, atom_count, _trace=False):
    N = int(atom_count)
    mag = _run_mlp(edge_emb, W1, b1, W2, b2, W3, b3, trace=_trace)
    return _forces_from_mag(mag, edge_vectors, edge_lengths, edge_index,
                            edge_cell_shift, N)


# revision 22
# speedup vs baseline: 1.0052x; 1.0050x over previous
"""DirectForce GNN message-passing kernel for 8 Trainium2 NeuronCores.

Structure
---------
Device (8 cores, edge-sharded, weights replicated):
    the edge MLP  mag_e = W3.(ssp(W2.(ssp(W1.x))))  for all E=262144 edges.
    The kernel is ACT-bound: softplus costs two table passes (exp, ln) per
    element on the scalar engine, which runs a flat 1 elem/cycle/partition
    regardless of dtype (measured), i.e. ~6.9us per 512-edge tile.  The
    design therefore minimizes ACT instruction count (4 wide instructions
    per tile over 4-bank PSUM tiles) and keeps every other engine under
    that roofline:

    * L1 runs feature-major: out = W1chunk.T @ x -> z1[feat, edge] in a
      single [128,4,512] PSUM tile (4 banks).  One EXP covers all 2048
      elements; the ShiftedSoftplus shift is folded into the LN pass as
      ln(0.5*e + 0.5) = ln(1+e^z) - ln 2  (scale/bias of the ACT op), so
      no bias folding on the host at all.
    * L2 runs TRANSPOSED: stationary operand is the h1 chunk, moving is
      W2, so z2 lands as [edge, feat] with edges on PSUM partitions.
      Again one 4-bank PSUM tile, one EXP, one LN.
    * L3 collapses to a per-partition dot: mag[e] = sum_j h2[e,j]*W3[j],
      one DVE scalar_tensor_tensor with accum_out per 128-edge chunk
      (sum over the free dim; b3 is added on the host).  No ones-matmul,
      no partition reduction, no staging copies.
    * Matmuls are fp16/bf16 (same PE rate as fp32r, half the DMA and
      LDWEIGHTS traffic); e-tiles are bf16 (exp can overflow fp16 range),
      h-tiles fp16.
    * Emission is software-pipelined one tile deep (L1(t+1) is issued
      between L1(t) and L2(t)) so the PE never waits on the softplus
      latency and both 4-bank PSUM pools (8 banks total) single-buffer
      without stalls.

Host (index work + O(E) reductions, ~0.3% of the FLOPs):
    the category/key lexsort pairing (exact transcription of the
    reference), magnitude symmetrization with the paired reverse edge,
    and the [N,3] segment-sum of mag * unit_vec over center atoms.

Hardware constraint that shapes the emission: every TPB instruction encodes
at most ONE semaphore wait (NEURON_ISA_TPB_EVENTS has a single wait slot).
Tile emits multi-wait instructions freely, so after scheduling we legalize:
every excess wait is hoisted onto a NOP inserted just before the offending
instruction on the same engine -- sound because each engine's sequencer
executes waits in program order.
"""

import numpy as np

E = 262144
D = 512
N_CORES = 8
RPC = E // N_CORES          # rows (edges) per core = 32768
RT = 512                    # rows per tile iteration
NT = RPC // RT              # 64 row-tiles per core
KC = D // 128               # 4 contraction chunks

_CACHE = {}


def _legalize_waits(nc):
    """Every TPB instruction carries at most one sync wait; hoist extras onto
    same-engine NOPs placed immediately before the offender."""
    import concourse.mybir as mybir

    eng_map = {
        mybir.EngineType.PE: nc.tensor,
        mybir.EngineType.Activation: nc.scalar,
        mybir.EngineType.DVE: nc.vector,
        mybir.EngineType.Pool: nc.gpsimd,
        mybir.EngineType.SP: nc.sync,
    }
    hoist_all = getattr(nc, "_hoist_all_wait_ids", frozenset())
    n_nops = 0
    for blk in nc.main_func.blocks:
        offenders = [
            ins for ins in blk.instructions
            if ins.sync_info is not None and (
                len(ins.sync_info.on_wait) > 1
                # Matmuls lower to LDWEIGHTS+MATMUL; if the wait rides on the
                # MATMUL, the LDWEIGHTS can read a still-being-written
                # stationary operand (the L2 matmuls' stationary is h1,
                # produced by the LN moments earlier).  For those, hoist ALL
                # waits onto preceding NOPs so they execute before the
                # weight load.
                or (len(ins.sync_info.on_wait) >= 1 and id(ins) in hoist_all)
            )
        ]
        for ins in offenders:
            si = ins.sync_info
            waits = list(si.on_wait)
            if id(ins) in hoist_all:
                si.on_wait = []
            else:
                si.on_wait = [waits[-1]]
                waits = waits[:-1]
            eng = eng_map.get(ins.engine, nc.sync)
            idx = blk.instructions.index(ins)
            for w in waits:
                nop_ins = eng.nop(nofuse=True).ins
                nop_ins.sync_info = mybir.SyncInfo(on_wait=[w], on_update=[])
                # nop() appended it to the current bb; move it before `ins`
                cur = nc.cur_bb.bb
                cur.instructions.remove(nop_ins)
                blk.instructions.insert(idx, nop_ins)
                idx += 1
                n_nops += 1
    return n_nops


def _build_program(zero_b1=True, zero_b2=True):
    import concourse.bass as bass
    import concourse.mybir as mybir
    import concourse.tile as tile

    f32 = mybir.dt.float32
    f16 = mybir.dt.float16
    bf16 = mybir.dt.bfloat16
    AF = mybir.ActivationFunctionType
    OP = mybir.AluOpType

    nc = bass.Bass()
    xt = nc.dram_tensor("xt", [D, RPC], f16, kind="ExternalInput")
    w1p = nc.dram_tensor("w1p", [128, KC, D], f16, kind="ExternalInput")
    w2p = nc.dram_tensor("w2p", [128, KC, D], f16, kind="ExternalInput")
    w3b = nc.dram_tensor("w3b", [128, D], f16, kind="ExternalInput")
    b1p = nc.dram_tensor("b1p", [128, KC], f32, kind="ExternalInput")
    b2r = nc.dram_tensor("b2r", [1, D], f16, kind="ExternalInput")
    # mag for edge  t*RT + c*128 + p  lives at mag[p, 4*t + c]
    mag = nc.dram_tensor("mag", [128, KC * NT], f32, kind="ExternalOutput")

    xt_v = xt.rearrange("(c p) r -> p c r", p=128)  # [128, KC, RPC]

    with tile.TileContext(nc) as tc:
        with (
            tc.tile_pool(name="singles", bufs=1) as singles,
            tc.tile_pool(name="xp", bufs=3) as xp,
            tc.tile_pool(name="e2p", bufs=2) as e2p,
            tc.tile_pool(name="e1p", bufs=2) as e1p,
            tc.tile_pool(name="h1p", bufs=2) as h1p,
            tc.tile_pool(name="h2p", bufs=2) as h2p,
            tc.tile_pool(name="prodp", bufs=2) as prodp,
            tc.tile_pool(name="ps1p", bufs=1, space="PSUM") as ps1p,
            tc.tile_pool(name="ps2p", bufs=1, space="PSUM") as ps2p,
        ):
            w1t = singles.tile([128, KC, D], f16)
            nc.sync.dma_start(out=w1t, in_=w1p[:, :, :])
            w2t = singles.tile([128, KC, D], f16)
            nc.sync.dma_start(out=w2t, in_=w2p[:, :, :])
            w3t = singles.tile([128, D], f16)
            nc.sync.dma_start(out=w3t, in_=w3b[:, :])
            half_t = singles.tile([128, 1], f32)
            nc.vector.memset(half_t, 0.5)
            if not zero_b1:
                b1t = singles.tile([128, KC], f32)
                nc.sync.dma_start(out=b1t, in_=b1p[:, :])
            if not zero_b2:
                b2t = singles.tile([1, D], f16)
                nc.sync.dma_start(out=b2t, in_=b2r[:, :])
                ones1 = singles.tile([1, 128], f16)
                nc.vector.memset(ones1, 1.0)
            magsb = singles.tile([128, KC * NT], f32)

            l2_ids = set()
            h1_prev = None
            for t in range(NT + 1):
                if t < NT:
                    # ---- L1(t): z1[feat, edge] = W1.T @ x
                    x_t = xp.tile([128, KC, RT], f16, tag="x")
                    nc.sync.dma_start(out=x_t,
                                      in_=xt_v[:, :, t * RT:(t + 1) * RT])
                    ps1 = ps1p.tile([128, KC, RT], f32, tag="ps1")
                    for jc in range(KC):
                        for dc in range(KC):
                            r = nc.tensor.matmul(
                                ps1[:, jc, :],
                                w1t[:, dc, jc * 128:(jc + 1) * 128],
                                x_t[:, dc, :],
                                start=(dc == 0), stop=(dc == KC - 1),
                            )
                            if t == 0 and jc == 0 and dc == 0:
                                # tile 0: the w1t DMA is still in flight; the
                                # first LDWEIGHTS must not outrun its wait
                                l2_ids.add(id(r.ins))
                    e1 = e1p.tile([128, KC, RT], bf16, tag="e1")
                    if zero_b1 and t == 0:
                        nc.scalar.activation(e1[:, 0:2, :], ps1[:, 0:2, :],
                                             AF.Exp)
                        nc.scalar.activation(e1[:, 2:4, :], ps1[:, 2:4, :],
                                             AF.Exp)
                    elif zero_b1:
                        nc.scalar.activation(e1, ps1, AF.Exp)
                    else:
                        for jc in range(KC):
                            nc.scalar.activation(e1[:, jc, :], ps1[:, jc, :],
                                                 AF.Exp, bias=b1t[:, jc:jc + 1])
                    h1 = h1p.tile([128, KC, RT], f16, tag="h1")
                    # ssp(z) = ln(0.5*e^z + 0.5)
                    nc.scalar.activation(h1, e1, AF.Ln, bias=half_t[:, 0:1], scale=0.5)

                if t >= 1:
                    # ---- L2(t-1), transposed: z2[edge, feat] = h1chunk.T @ W2
                    tm = t - 1
                    h1m = h1_prev
                    ps2 = ps2p.tile([128, KC, RT], f32, tag="ps2")
                    for c in range(KC):
                        if not zero_b2:
                            nc.tensor.matmul(ps2[:, c, :], ones1, b2t,
                                             start=True, stop=False)
                        for kc in range(KC):
                            r = nc.tensor.matmul(
                                ps2[:, c, :],
                                h1m[:, kc, c * 128:(c + 1) * 128],
                                w2t[:, kc, :],
                                start=(kc == 0 and zero_b2),
                                stop=(kc == KC - 1),
                            )
                            # Only the first matmul of the L2 block can have
                            # its LDWEIGHTS outrun the h1 producer: the shadow
                            # weight buffer is depth-1, so every later LDW
                            # issues after the previous instruction (and hence
                            # after the hoisted wait) has started.
                            if c == 0 and kc == 0:
                                l2_ids.add(id(r.ins))
                    e2 = e2p.tile([128, KC, RT], bf16, tag="e2")
                    h2 = h2p.tile([128, KC, RT], f16, tag="h2")
                    if tm == NT - 1:
                        # drain tile: quarter EXPs start after each c-group's
                        # 4 matmuls instead of all 16, halved LNs release the
                        # dots sooner -- shortens the tail by ~3 us
                        for c in range(KC):
                            nc.scalar.activation(e2[:, c, :], ps2[:, c, :],
                                                 AF.Exp)
                        nc.scalar.activation(h2[:, 0:2, :], e2[:, 0:2, :],
                                             AF.Ln, bias=half_t[:, 0:1],
                                             scale=0.5)
                        nc.scalar.activation(h2[:, 2:4, :], e2[:, 2:4, :],
                                             AF.Ln, bias=half_t[:, 0:1],
                                             scale=0.5)
                    else:
                        nc.scalar.activation(e2, ps2, AF.Exp)
                        nc.scalar.activation(h2, e2, AF.Ln,
                                             bias=half_t[:, 0:1], scale=0.5)
                    # ---- L3: mag[e] = sum_j h2[e, j] * W3[j]  (b3 on host)
                    for c in range(KC):
                        prod = prodp.tile([128, RT], f16, tag="prod")
                        nc.vector.scalar_tensor_tensor(
                            prod, h2[:, c, :], 1.0, w3t,
                            OP.mult, OP.mult,
                            accum_out=magsb[:, KC * tm + c:KC * tm + c + 1],
                        )

                if t < NT:
                    h1_prev = h1

            nc.sync.dma_start(out=mag[:, :], in_=magsb)

    nc._hoist_all_wait_ids = frozenset(l2_ids)
    _legalize_waits(nc)
    return nc


def _get_program(zero_b1=True, zero_b2=True):
    key = (zero_b1, zero_b2)
    if key not in _CACHE:
        _CACHE[key] = _build_program(zero_b1, zero_b2)
    return _CACHE[key]


def _run_mlp(edge_emb, W1, b1, W2, b2, W3, b3, trace=False):
    """Run the edge MLP on 8 NeuronCores; returns mag [E] fp32 (incl. b3)."""
    from concourse.bass_utils import run_bass_kernel_spmd

    W1 = np.asarray(W1, np.float32)
    W2 = np.asarray(W2, np.float32)
    W3 = np.asarray(W3, np.float32)
    b1 = np.asarray(b1, np.float32)
    b2 = np.asarray(b2, np.float32)
    b3 = np.asarray(b3, np.float32)

    zb1 = not np.any(b1)
    zb2 = not np.any(b2)
    nc = _get_program(zb1, zb2)

    # w1p[p, dc, j] = W1[dc*128+p, j];  w2p[p, kc, j] = W2[kc*128+p, j]
    w1p = np.ascontiguousarray(
        W1.reshape(KC, 128, D).transpose(1, 0, 2).astype(np.float16))
    w2p = np.ascontiguousarray(
        W2.reshape(KC, 128, D).transpose(1, 0, 2).astype(np.float16))
    w3bc = np.ascontiguousarray(
        np.broadcast_to(W3[:, 0].astype(np.float16)[None, :], (128, D)))
    b1pk = np.ascontiguousarray(b1.reshape(KC, 128).T.astype(np.float32))
    b2rw = b2.astype(np.float16)[None, :]

    emb = np.asarray(edge_emb, np.float32)
    in_maps = []
    for c in range(N_CORES):
        shard = emb[c * RPC:(c + 1) * RPC, :]
        xt_shard = np.ascontiguousarray(shard.T).astype(np.float16)
        in_maps.append({"xt": xt_shard, "w1p": w1p, "w2p": w2p, "w3b": w3bc,
                        "b1p": b1pk, "b2r": b2rw})

    kwargs = {}
    if trace:
        _register_ntff_hook()
        kwargs["trace"] = True
    res = run_bass_kernel_spmd(nc, in_maps, core_ids=list(range(N_CORES)),
                               **kwargs)
    shards = []
    for c in range(N_CORES):
        m = res.results[c]["mag"]            # [128, KC*NT], col = 4*t + c
        # edge t*RT + c*128 + p  ->  [t, c, p] order
        shards.append(np.ascontiguousarray(
            m.reshape(128, NT, KC).transpose(1, 2, 0)).reshape(-1))
    mag_out = np.concatenate(shards)
    if trace:
        print(f"HW exec time: {res.exec_time_ns} ns "
              f"(mean {res.mean_exec_time_ns} ns across cores)")
    return mag_out + np.float32(b3[0])


def _register_ntff_hook():
    """The image's antenv lacks axon_hooks; synthesize it so trace=True can
    capture NTFF profiles through the axon PJRT library."""
    import sys, types
    if "antenv.axon_hooks" in sys.modules:
        return
    mod = types.ModuleType("antenv.axon_hooks")
    state = {"hook": None}
    mod.set_axon_ntff_profile_hook = lambda h: state.__setitem__("hook", h)
    mod.get_axon_ntff_profile_hook = lambda: state["hook"]
    sys.modules["antenv.axon_hooks"] = mod
    import antenv
    antenv.axon_hooks = mod
    try:
        from trn_agent_boot.trn_boot import _ntff_profile_via_ctypes
        mod.set_axon_ntff_profile_hook(
            _ntff_profile_via_ctypes("/opt/axon/libaxon_pjrt.so"))
    except Exception:
        pass


def _forces_from_mag(mag, edge_vectors, edge_lengths, edge_index,
                     edge_cell_shift, N):
    """Exact numpy transcription of the reference pairing + segment sum."""
    uv = np.asarray(edge_vectors, np.float32) / np.asarray(
        edge_lengths, np.float32)[:, None]
    s = np.asarray(edge_cell_shift, np.int64)
    s0, s1, s2 = s[:, 0], s[:, 1], s[:, 2]
    c = np.asarray(edge_index[0], np.int64)
    n = np.asarray(edge_index[1], np.int64)
    fwd = c * N + n
    rev = n * N + c
    N2 = N * N
    conds = [
        (s0 == 0) & (s1 == 0) & (s2 == 0),
        (s0 == -1) & (s1 == 0) & (s2 == 0),
        (s1 == -1) & (s2 == 0),
        (s2 == -1),
        (s0 == 1) & (s1 == 0) & (s2 == 0),
        (s1 == 1) & (s2 == 0),
        (s2 == 1),
    ]
    keys = [
        fwd,
        fwd,
        (s0 + 2) * N2 + fwd,
        (s0 + 6) * (s1 + 2) * N2 + fwd,
        rev,
        (-s0 + 2) * N2 + rev,
        (-s0 + 6) * (-s1 + 2) * N2 + rev,
    ]
    cat = np.select(conds, [np.full_like(c, i) for i in range(7)],
                    np.full_like(c, 6))
    key = np.select(conds, keys, rev)
    perm = np.lexsort((key, cat))
    mag_s = mag[perm]
    uv_s = uv[perm]
    c_s = c[perm]
    n_s = n[perm]
    cat_s = cat[perm]
    perm2 = np.lexsort((n_s * N + c_s, cat_s))
    M = int(np.sum((cat_s >= 1) & (cat_s <= 3)))
    idx = np.arange(E, dtype=np.int64)
    partner = np.where(cat_s == 0, perm2,
                       np.where(cat_s <= 3, idx + M, idx - M))
    mag_f = (mag_s + mag_s[partner]) * np.float32(0.5)
    contrib = mag_f[:, None] * uv_s
    forces = np.empty((N, 3), np.float32)
    for d in range(3):
        forces[:, d] = np.bincount(c_s, weights=contrib[:, d],
                                   minlength=N).astype(np.float32)
    return forces


def kernel(edge_emb, edge_vectors, edge_lengths, W1, b1, W2, b2, W3, b3,
           edge_index, edge_cell_shift, atom_count, _trace=False):
    N = int(atom_count)
    mag = _run_mlp(edge_emb, W1, b1, W2, b2, W3, b3, trace=_trace)
    return _forces_from_mag(mag, edge_vectors, edge_lengths, edge_index,
                            edge_cell_shift, N)
